# revision 23
# baseline (speedup 1.0000x reference)
"""GATv2Conv-with-edge-features Trainium2 kernel (8-core SPMD, edge-sharded by dst).

Self-contained: hardcodes problem shapes (N=50000 nodes, E=800000 edges,
128 feat, 8 heads x 16). Sharding: core k owns dst nodes [6250k, 6250(k+1))
and all edges pointing into that range. Within a core, edges are grouped by
dst segment and packed into tiles of <=128 edges spanning <=32 consecutive
dst nodes, so the per-dst softmax + scatter-sum reduce entirely on-chip via
a one-hot matmul per tile — no collectives, no atomic scatters, and no
device-side gathers in the hot loop (x[src], x[dst], efeat are host-staged
per edge slot in bf16, feature-major).

Device data flow per 2048-edge super-tile (16 tiles of 128 edges):
  T(psum) = x[src]@Ws.T + x[dst]@Wd.T + efeat@We.T   (3 accumulating bf16
            matmuls per tile, fp32 PSUM)
  L       = max(T, 0.2*T); score = reduce_d(L * attn); ex = exp(score)
  msg     = T * ex               (bf16)
  U,z     = S.T @ [msg, ex]      (S one-hot, host-built bf16) -> DRAM slots
Per node n (slot map gather): out = relu(U/z - feat_dst[n]), using
sum_e alpha*(fs+fe) = sum_e alpha*T - feat_dst (alpha sums to 1 per dst).
Softmax runs without max-subtraction (scores are O(+-10) here; exp is safe
in fp32).
"""
import numpy as np
import ml_dtypes

import concourse.bacc as bacc
import concourse.bass as bass
import concourse.tile as tile
import concourse.mybir as mybir
from concourse.bass import IndirectOffsetOnAxis
from concourse.bass_utils import run_bass_kernel_spmd

N_NODES = 50000
N_CORES = 8
N_LOCAL = N_NODES // N_CORES          # 6250
IN_FEAT = 128
HEADS = 8
HEAD_DIM = 16
NEG_SLOPE = 0.2
TILE_E = 128                          # edges per tile
TILE_W = 32                           # dst-node window per tile
ST_TILES = 16                         # tiles per super-tile
EDGE_BLK = ST_TILES * TILE_E          # 2048 edge slots per super-tile
EPS_Z = 1e-12                         # DVE reciprocal valid to ~2^-42
P = 128
FP = mybir.dt.float32
BF = mybir.dt.float16
I32 = mybir.dt.int32
BF_NP = np.float16


# ---------------------------------------------------------------- host prep

def _pack_core(dst_local, deg, n_local):
    order = np.argsort(dst_local, kind="stable")
    tile_base, tile_cnt, tile_w = [], [], []
    node_slot = np.full(n_local, -1, np.int64)
    cur_base = cur_cnt = cur_w = 0
    t = 0
    started = False
    empties = []

    def close():
        nonlocal t
        tile_base.append(cur_base)
        tile_cnt.append(cur_cnt)
        tile_w.append(cur_w)
        t += 1

    for n in range(n_local):
        d = int(deg[n])
        if d == 0:
            empties.append(n)
            continue
        assert d <= TILE_E, f"node degree {d} exceeds tile capacity {TILE_E}"
        if not started:
            cur_base, cur_cnt, cur_w = n, 0, 0
            started = True
        if cur_cnt + d > TILE_E or (n - cur_base) >= TILE_W:
            close()
            cur_base, cur_cnt, cur_w = n, 0, 0
        node_slot[n] = t * TILE_W + (n - cur_base)
        cur_w = n - cur_base + 1
        cur_cnt += d
    if started:
        close()

    free_slots = []
    for ti in range(t):
        for s in range(tile_w[ti], TILE_W):
            free_slots.append(ti * TILE_W + s)
    fi = 0
    for n in empties:
        if fi >= len(free_slots):
            tile_base.append(0)
            tile_cnt.append(0)
            tile_w.append(0)
            for s in range(TILE_W):
                free_slots.append(t * TILE_W + s)
            t += 1
        node_slot[n] = free_slots[fi]
        fi += 1
    assert (node_slot >= 0).all()
    return order, np.asarray(tile_base), np.asarray(tile_cnt), node_slot


def _prep_cores(x, efeat, src, dst, W_src, b_src, W_dst, b_dst, W_edge, attn):
    x = np.ascontiguousarray(np.asarray(x, np.float32))
    efeat = np.asarray(efeat, np.float32)
    src = np.asarray(src).astype(np.int64)
    dst = np.asarray(dst).astype(np.int64)
    W_src = np.asarray(W_src, np.float32)
    W_dst = np.asarray(W_dst, np.float32)
    W_edge = np.asarray(W_edge, np.float32)
    b_src = np.asarray(b_src, np.float32)
    b_dst = np.asarray(b_dst, np.float32)
    attn = np.asarray(attn, np.float32)
    has_bias = bool(max(np.abs(b_src).max(), np.abs(b_dst).max()) > 0)

    per_core = []
    core_T = []
    for k in range(N_CORES):
        lo = k * N_LOCAL
        eidx = np.nonzero((dst >= lo) & (dst < lo + N_LOCAL))[0]
        dl = dst[eidx] - lo
        deg = np.bincount(dl, minlength=N_LOCAL)
        order, tb, tcnt, node_slot = _pack_core(dl, deg, N_LOCAL)
        per_core.append((eidx[order], dl[order], tb, tcnt, node_slot))
        core_T.append(len(tb))

    T_tiles = max(core_T)
    T_tiles = ((T_tiles + ST_TILES - 1) // ST_TILES) * ST_TILES
    E_pad = T_tiles * TILE_E

    x16 = x.astype(BF_NP)
    ef16 = efeat.astype(BF_NP)
    attn_rep = np.ascontiguousarray(
        np.broadcast_to(attn.reshape(1, HEADS * HEAD_DIM), (P, HEADS * HEAD_DIM)))
    bias_sum = np.ascontiguousarray(
        np.broadcast_to((b_src + b_dst)[None, :], (P, IN_FEAT)))
    bdst_rep = np.ascontiguousarray(np.broadcast_to(b_dst[None, :], (P, IN_FEAT)))
    n_fin = (N_LOCAL + P - 1) // P * P

    in_maps = []
    for k in range(N_CORES):
        eidx, dl, tb, tcnt, node_slot = per_core[k]
        lo = k * N_LOCAL

        src_ids = np.zeros((TILE_E, T_tiles), np.int64)
        fd_ids = np.zeros((TILE_E, T_tiles), np.int64)
        slot_reb = np.full((TILE_E, T_tiles), -1, np.int64)
        ef_ids = np.full((TILE_E, T_tiles), -1, np.int64)
        pos = 0
        for t in range(len(tb)):
            c = int(tcnt[t])
            if c == 0:
                continue
            e_ids = eidx[pos:pos + c]
            d_loc = dl[pos:pos + c]
            pos += c
            src_ids[:c, t] = src[e_ids]
            fd_ids[:c, t] = d_loc + lo
            slot_reb[:c, t] = d_loc - tb[t]
            ef_ids[:c, t] = e_ids
        assert pos == len(eidx)

        # staged per-edge features, feature-major [128, E_pad], col = t*128+p
        xsrcT = np.ascontiguousarray(x16[src_ids.T.ravel()].T)
        xdstT = np.ascontiguousarray(x16[fd_ids.T.ravel()].T)
        ce = ef_ids.T.ravel()
        efTv = ef16[np.maximum(ce, 0)]
        efTv[ce < 0] = 0
        efTv = np.ascontiguousarray(efTv.T)

        S_all = np.zeros((TILE_E, T_tiles * TILE_W), BF_NP)
        pp, tt_ = np.nonzero(slot_reb >= 0)
        S_all[pp, tt_ * TILE_W + slot_reb[pp, tt_]] = 1.0

        sm = np.zeros(n_fin, np.int32)
        sm[:N_LOCAL] = node_slot.astype(np.int32)
        slot_map = np.ascontiguousarray(sm.reshape(n_fin // P, P).T)

        in_maps.append(dict(
            xT_local=np.ascontiguousarray(x[k * N_LOCAL:(k + 1) * N_LOCAL].T),
            W_dstT32=np.ascontiguousarray(W_dst.T),
            Ws16=np.ascontiguousarray(W_src.T.astype(BF_NP)),
            Wd16=np.ascontiguousarray(W_dst.T.astype(BF_NP)),
            We16=np.ascontiguousarray(W_edge.T.astype(BF_NP)),
            bias_sum=bias_sum,
            bdst_rep=bdst_rep,
            attn_rep=attn_rep,
            xsrcT=xsrcT,
            xdstT=xdstT,
            efT=efTv,
            S_all=S_all,
            slot_map=slot_map,
        ))
    return in_maps, T_tiles, has_bias


# ------------------------------------------------------------- bass program

def _chunks(total, step):
    out = []
    c0 = 0
    while c0 < total:
        out.append((c0, min(step, total - c0)))
        c0 += step
    return out


def build_program(T_tiles, has_bias=False, dbg=False):
    nc = bacc.Bacc("TRN2", target_bir_lowering=False, debug=False,
                   num_devices=N_CORES)
    ikind = "ExternalOutput" if dbg else "Internal"
    n_st = T_tiles // ST_TILES
    n_fin = (N_LOCAL + P - 1) // P * P

    xTl_d = nc.dram_tensor("xT_local", [IN_FEAT, N_LOCAL], FP, kind="ExternalInput")
    WdT32_d = nc.dram_tensor("W_dstT32", [IN_FEAT, IN_FEAT], FP, kind="ExternalInput")
    Ws16_d = nc.dram_tensor("Ws16", [IN_FEAT, IN_FEAT], BF, kind="ExternalInput")
    Wd16_d = nc.dram_tensor("Wd16", [IN_FEAT, IN_FEAT], BF, kind="ExternalInput")
    We16_d = nc.dram_tensor("We16", [IN_FEAT, IN_FEAT], BF, kind="ExternalInput")
    bsum_d = nc.dram_tensor("bias_sum", [P, IN_FEAT], FP, kind="ExternalInput")
    bdst_d = nc.dram_tensor("bdst_rep", [P, IN_FEAT], FP, kind="ExternalInput")
    attn_d = nc.dram_tensor("attn_rep", [P, IN_FEAT], FP, kind="ExternalInput")
    xsT_d = nc.dram_tensor("xsrcT", [IN_FEAT, T_tiles * TILE_E], BF,
                           kind="ExternalInput")
    xdT_d = nc.dram_tensor("xdstT", [IN_FEAT, T_tiles * TILE_E], BF,
                           kind="ExternalInput")
    efT_d = nc.dram_tensor("efT", [IN_FEAT, T_tiles * TILE_E], BF,
                           kind="ExternalInput")
    S_d = nc.dram_tensor("S_all", [TILE_E, T_tiles * TILE_W], BF,
                         kind="ExternalInput")
    smap_d = nc.dram_tensor("slot_map", [P, n_fin // P], I32, kind="ExternalInput")

    fdst_d = nc.dram_tensor("feat_dst_i", [N_LOCAL, IN_FEAT], FP, kind=ikind)
    U_d = nc.dram_tensor("U_i", [T_tiles * TILE_W, IN_FEAT], FP, kind=ikind)
    z_d = nc.dram_tensor("z_i", [T_tiles * TILE_W, HEADS], FP, kind=ikind)
    out_d = nc.dram_tensor("out", [N_LOCAL, IN_FEAT], FP, kind="ExternalOutput")

    HB = EDGE_BLK // 2   # 1024, psum half block

    with tile.TileContext(nc) as tc:
        with tc.tile_pool(name="const", bufs=1) as cb:
            WdT32 = cb.tile([P, IN_FEAT], FP)
            nc.sync.dma_start(out=WdT32[:], in_=WdT32_d[:])
            Ws16 = cb.tile([P, IN_FEAT], BF)
            nc.sync.dma_start(out=Ws16[:], in_=Ws16_d[:])
            Wd16 = cb.tile([P, IN_FEAT], BF)
            nc.sync.dma_start(out=Wd16[:], in_=Wd16_d[:])
            We16 = cb.tile([P, IN_FEAT], BF)
            nc.sync.dma_start(out=We16[:], in_=We16_d[:])
            bsum = cb.tile([P, IN_FEAT], FP)
            nc.sync.dma_start(out=bsum[:], in_=bsum_d[:])
            bdst = cb.tile([P, IN_FEAT], FP)
            nc.sync.dma_start(out=bdst[:], in_=bdst_d[:])
            attn_sb = cb.tile([P, IN_FEAT], FP)
            nc.sync.dma_start(out=attn_sb[:], in_=attn_d[:])
            smap_sb = cb.tile([P, n_fin // P], I32)
            nc.sync.dma_start(out=smap_sb[:], in_=smap_d[:])

            # ---------------- phase A: feat_dst projection (+ b_dst)
            with (
                tc.tile_pool(name="pa_sb", bufs=3) as pas,
                tc.tile_pool(name="pa_ps", bufs=4, space="PSUM") as pap,
                tc.tile_pool(name="pa_tr", bufs=1, space="PSUM") as patr,
            ):
                for c0, cw in _chunks(N_LOCAL, 2048):
                    xc = pas.tile([P, 2048], FP, tag="xc")
                    nc.sync.dma_start(out=xc[:, :cw], in_=xTl_d[:, c0:c0 + cw])
                    tr = patr.tile([TILE_W, 1], FP, tag="trash")
                    nc.tensor.matmul(out=tr[:], lhsT=xc[:, :TILE_W],
                                     rhs=xc[:, :1], start=True, stop=True)
                    for j0, jw in _chunks(cw, P):
                        ps = pap.tile([P, IN_FEAT], FP, tag="pa")
                        nc.tensor.matmul(out=ps[:jw, :], lhsT=xc[:, j0:j0 + jw],
                                         rhs=WdT32[:], start=True, stop=True)
                        ev = pas.tile([P, IN_FEAT], FP, tag="ev")
                        nc.vector.tensor_tensor(out=ev[:jw, :], in0=ps[:jw, :],
                                                in1=bdst[:jw, :],
                                                op=mybir.AluOpType.add)
                        nc.sync.dma_start(out=fdst_d[c0 + j0:c0 + j0 + jw, :],
                                          in_=ev[:jw, :])

            with tc.tile_critical():
                nc.all_engine_barrier()

            # ---------------- phase B: edge super-tiles
            with (
                tc.tile_pool(name="eb_sb", bufs=2) as eb,
                tc.tile_pool(name="eb_ps", bufs=2, space="PSUM") as ep,
                tc.tile_pool(name="eb_ps1", bufs=1, space="PSUM") as ep1,
            ):
                for st in range(n_st):
                    e0 = st * EDGE_BLK

                    xsc = eb.tile([P, EDGE_BLK], BF, tag="xsc")
                    nc.sync.dma_start(out=xsc[:], in_=xsT_d[:, e0:e0 + EDGE_BLK])
                    xdc = eb.tile([P, EDGE_BLK], BF, tag="xdc")
                    nc.sync.dma_start(out=xdc[:], in_=xdT_d[:, e0:e0 + EDGE_BLK])
                    efc = eb.tile([P, EDGE_BLK], BF, tag="efc")
                    nc.sync.dma_start(out=efc[:], in_=efT_d[:, e0:e0 + EDGE_BLK])
                    S_sb = eb.tile([TILE_E, ST_TILES * TILE_W], BF, tag="S")
                    nc.sync.dma_start(
                        out=S_sb[:],
                        in_=S_d[:, st * ST_TILES * TILE_W:(st + 1) * ST_TILES * TILE_W])

                    # wait-absorbing dummies (PE matmuls allow only one wait)
                    tre = ep1.tile([TILE_W, 1], FP, tag="trash")
                    nc.tensor.matmul(out=tre[:], lhsT=xsc[:, :TILE_W],
                                     rhs=xsc[:, :1], start=True, stop=True)
                    nc.tensor.matmul(out=tre[:], lhsT=xdc[:, :TILE_W],
                                     rhs=xdc[:, :1], start=True, stop=True)
                    nc.tensor.matmul(out=tre[:], lhsT=efc[:, :TILE_W],
                                     rhs=efc[:, :1], start=True, stop=True)
                    nc.tensor.matmul(out=tre[:], lhsT=S_sb[:, :TILE_W],
                                     rhs=S_sb[:, :1], start=True, stop=True)

                    # T = fs + fd + fe into PSUM (3 accumulating matmuls/tile)
                    halves = []
                    for h in range(2):
                        psh = ep.tile([P, HB], FP, tag="ps")
                        for tt in range(8):
                            c = (h * 8 + tt) * TILE_E
                            sl = slice(tt * TILE_E, (tt + 1) * TILE_E)
                            nc.tensor.matmul(out=psh[:, sl], lhsT=xsc[:, c:c + TILE_E],
                                             rhs=Ws16[:], start=True, stop=False)
                            nc.tensor.matmul(out=psh[:, sl], lhsT=xdc[:, c:c + TILE_E],
                                             rhs=Wd16[:], start=False, stop=False)
                            nc.tensor.matmul(out=psh[:, sl], lhsT=efc[:, c:c + TILE_E],
                                             rhs=We16[:], start=False, stop=True)
                        halves.append(psh)

                    if has_bias:
                        Tt = eb.tile([P, EDGE_BLK], FP, tag="Tt")
                        for h in range(2):
                            hs = slice(h * HB, (h + 1) * HB)
                            nc.vector.tensor_tensor(
                                out=Tt[:, hs].rearrange("p (t f) -> p t f", t=8),
                                in0=halves[h][:].rearrange("p (t f) -> p t f", t=8),
                                in1=bsum[:].unsqueeze(1).to_broadcast([P, 8, IN_FEAT]),
                                op=mybir.AluOpType.add)
                        tsrc = [Tt[:, :HB], Tt[:, HB:]]
                    else:
                        tsrc = [halves[0][:], halves[1][:]]

                    # leaky-relu then * attn
                    T2 = eb.tile([P, EDGE_BLK], FP, tag="T2")
                    for h in range(2):
                        hs = slice(h * HB, (h + 1) * HB)
                        nc.scalar.activation(out=T2[:, hs], in_=tsrc[h],
                                             func=mybir.ActivationFunctionType.Copy,
                                             scale=NEG_SLOPE)
                        nc.vector.tensor_tensor(out=T2[:, hs], in0=tsrc[h],
                                                in1=T2[:, hs],
                                                op=mybir.AluOpType.max)
                    attn_b = attn_sb[:].unsqueeze(1).to_broadcast(
                        [P, ST_TILES, IN_FEAT])
                    nc.vector.tensor_tensor(
                        out=T2[:].rearrange("p (t f) -> p t f", t=ST_TILES),
                        in0=T2[:].rearrange("p (t f) -> p t f", t=ST_TILES),
                        in1=attn_b, op=mybir.AluOpType.mult)

                    score = eb.tile([P, ST_TILES * HEADS], FP, tag="score")
                    nc.vector.tensor_reduce(
                        out=score[:],
                        in_=T2[:].rearrange("p (t h d) -> p t h d",
                                            h=HEADS, d=HEAD_DIM),
                        axis=mybir.AxisListType.X, op=mybir.AluOpType.add)
                    ex = eb.tile([P, ST_TILES * HEADS], BF, tag="ex")
                    nc.scalar.activation(out=ex[:], in_=score[:],
                                         func=mybir.ActivationFunctionType.Exp)

                    # msg = T * ex  (bf16)
                    msg = eb.tile([P, EDGE_BLK], BF, tag="msg")
                    for h in range(2):
                        hs = slice(h * HB, (h + 1) * HB)
                        ex_b = ex[:, h * 64:(h + 1) * 64] \
                            .rearrange("p (t hh) -> p t hh", hh=HEADS) \
                            .unsqueeze(3).to_broadcast([P, 8, HEADS, HEAD_DIM])
                        nc.vector.tensor_tensor(
                            out=msg[:, hs].rearrange("p (t hh d) -> p t hh d",
                                                     hh=HEADS, d=HEAD_DIM),
                            in0=tsrc[h].rearrange("p (t hh d) -> p t hh d",
                                                  hh=HEADS, d=HEAD_DIM),
                            in1=ex_b, op=mybir.AluOpType.mult)

                    # absorb DVE wait before scatter matmuls
                    nc.tensor.matmul(out=tre[:], lhsT=msg[:, :TILE_W],
                                     rhs=msg[:, :1], start=True, stop=True)

                    U_sb = eb.tile([TILE_W, ST_TILES * IN_FEAT], FP, tag="Usb")
                    z_ps = ep1.tile([TILE_W, ST_TILES * HEADS], FP, tag="zps")
                    for q in range(4):
                        U_ps = ep.tile([TILE_W, 4 * IN_FEAT], FP, tag="Ups")
                        for j in range(4):
                            tt = q * 4 + j
                            nc.tensor.matmul(
                                out=U_ps[:, j * IN_FEAT:(j + 1) * IN_FEAT],
                                lhsT=S_sb[:, tt * TILE_W:(tt + 1) * TILE_W],
                                rhs=msg[:, tt * TILE_E:(tt + 1) * TILE_E],
                                start=True, stop=True)
                            nc.tensor.matmul(
                                out=z_ps[:, tt * HEADS:(tt + 1) * HEADS],
                                lhsT=S_sb[:, tt * TILE_W:(tt + 1) * TILE_W],
                                rhs=ex[:, tt * HEADS:(tt + 1) * HEADS],
                                start=True, stop=True)
                        nc.scalar.activation(
                            out=U_sb[:, q * 512:(q + 1) * 512], in_=U_ps[:],
                            func=mybir.ActivationFunctionType.Copy)
                    z_sb = eb.tile([TILE_W, ST_TILES * HEADS], FP, tag="zsb")
                    nc.scalar.activation(out=z_sb[:], in_=z_ps[:],
                                         func=mybir.ActivationFunctionType.Copy)

                    nc.sync.dma_start(
                        out=U_d[st * ST_TILES * TILE_W:(st + 1) * ST_TILES * TILE_W, :]
                        .rearrange("(t w) f -> w t f", t=ST_TILES),
                        in_=U_sb[:].rearrange("p (t f) -> p t f", t=ST_TILES))
                    nc.sync.dma_start(
                        out=z_d[st * ST_TILES * TILE_W:(st + 1) * ST_TILES * TILE_W, :]
                        .rearrange("(t w) h -> w t h", t=ST_TILES),
                        in_=z_sb[:].rearrange("p (t h) -> p t h", t=ST_TILES))

            with tc.tile_critical():
                nc.all_engine_barrier()

            # ---------------- phase C: normalize, subtract feat_dst, relu
            with tc.tile_pool(name="fin", bufs=3) as fb:
                for i, (c0, w) in enumerate(_chunks(N_LOCAL, P)):
                    Ug = fb.tile([P, IN_FEAT], FP, tag="Ug")
                    nc.gpsimd.indirect_dma_start(
                        out=Ug[:w, :], out_offset=None, in_=U_d[:],
                        in_offset=IndirectOffsetOnAxis(
                            ap=smap_sb[:w, i:i + 1], axis=0))
                    zg = fb.tile([P, HEADS], FP, tag="zg")
                    nc.gpsimd.indirect_dma_start(
                        out=zg[:w, :], out_offset=None, in_=z_d[:],
                        in_offset=IndirectOffsetOnAxis(
                            ap=smap_sb[:w, i:i + 1], axis=0))
                    fdr = fb.tile([P, IN_FEAT], FP, tag="fdr")
                    nc.sync.dma_start(out=fdr[:w, :], in_=fdst_d[c0:c0 + w, :])

                    zs = fb.tile([P, HEADS], FP, tag="zs")
                    nc.vector.tensor_scalar_max(out=zs[:w, :], in0=zg[:w, :],
                                                scalar1=EPS_Z)
                    zr = fb.tile([P, HEADS], FP, tag="zr")
                    nc.vector.reciprocal(out=zr[:w, :], in_=zs[:w, :])
                    m = fb.tile([P, HEADS], FP, tag="m")
                    nc.vector.tensor_scalar(out=m[:w, :], in0=zg[:w, :],
                                            scalar1=0.0, scalar2=None,
                                            op0=mybir.AluOpType.is_gt)
                    mz = fb.tile([P, HEADS], FP, tag="mz")
                    nc.vector.tensor_tensor(out=mz[:w, :], in0=zr[:w, :],
                                            in1=m[:w, :], op=mybir.AluOpType.mult)
                    hp = fb.tile([P, IN_FEAT], FP, tag="hp")
                    mz_b = mz[:w, :].unsqueeze(2).to_broadcast([w, HEADS, HEAD_DIM])
                    nc.vector.tensor_tensor(
                        out=hp[:w, :].rearrange("p (h d) -> p h d", d=HEAD_DIM),
                        in0=Ug[:w, :].rearrange("p (h d) -> p h d", d=HEAD_DIM),
                        in1=mz_b, op=mybir.AluOpType.mult)
                    fdm = fb.tile([P, IN_FEAT], FP, tag="fdm")
                    m_b = m[:w, :].unsqueeze(2).to_broadcast([w, HEADS, HEAD_DIM])
                    nc.vector.tensor_tensor(
                        out=fdm[:w, :].rearrange("p (h d) -> p h d", d=HEAD_DIM),
                        in0=fdr[:w, :].rearrange("p (h d) -> p h d", d=HEAD_DIM),
                        in1=m_b, op=mybir.AluOpType.mult)
                    h2 = fb.tile([P, IN_FEAT], FP, tag="h2")
                    nc.vector.tensor_tensor(out=h2[:w, :], in0=hp[:w, :],
                                            in1=fdm[:w, :],
                                            op=mybir.AluOpType.subtract)
                    ob = fb.tile([P, IN_FEAT], FP, tag="ob")
                    nc.scalar.activation(out=ob[:w, :], in_=h2[:w, :],
                                         func=mybir.ActivationFunctionType.Relu)
                    nc.sync.dma_start(out=out_d[c0:c0 + w, :], in_=ob[:w, :])
    nc.compile()
    return nc


_PROGRAM_CACHE = {}


def kernel(**inputs) -> np.ndarray:
    in_maps, T_tiles, has_bias = _prep_cores(**inputs)
    key = (T_tiles, has_bias)
    if key not in _PROGRAM_CACHE:
        _PROGRAM_CACHE[key] = build_program(T_tiles, has_bias=has_bias)
    nc = _PROGRAM_CACHE[key]
    res = run_bass_kernel_spmd(nc, in_maps, list(range(N_CORES)))
    out = np.concatenate([np.asarray(res.results[k]["out"])
                          for k in range(N_CORES)], axis=0)
    return out.astype(np.float32)


if __name__ == "__main__":
    from prep import load_inputs_npz, reference_np
    inputs = load_inputs_npz()
    actual = kernel(**inputs)
    ref_in = {k: (v.astype(np.int64) if k in ("src", "dst")
                  else np.asarray(v, np.float32)) for k, v in inputs.items()}
    expected = reference_np(**ref_in)
    rel = np.linalg.norm(actual - expected) / np.linalg.norm(expected)
    print(f"rel l2 err: {rel:.3e}  max abs: {np.abs(actual - expected).max():.3e}")


# revision 25
# speedup vs baseline: 104.6657x; 104.6657x over previous
"""GATv2Conv-with-edge-features Trainium2 kernel (8-core SPMD, edge-sharded by dst).

Self-contained: hardcodes problem shapes (N=50000 nodes, E=800000 edges,
128 feat, 8 heads x 16). Sharding: core k owns dst nodes [6250k, 6250(k+1))
and all edges pointing into that range. Within a core, edges are grouped by
dst segment and packed into tiles of <=128 edges spanning <=32 consecutive
dst nodes, so the per-dst softmax + scatter-sum reduce entirely on-chip via
a one-hot matmul per tile — no collectives, no atomic scatters, and no
device-side gathers in the hot loop (x[src], x[dst], efeat are host-staged
per edge slot in bf16, feature-major).

Device data flow per 2048-edge super-tile (16 tiles of 128 edges):
  T(psum) = x[src]@Ws.T + x[dst]@Wd.T + efeat@We.T   (3 accumulating bf16
            matmuls per tile, fp32 PSUM)
  L       = max(T, 0.2*T); score = reduce_d(L * attn); ex = exp(score)
  msg     = T * ex               (bf16)
  U,z     = S.T @ [msg, ex]      (S one-hot, host-built bf16) -> DRAM slots
Per node n (slot map gather): out = relu(U/z - feat_dst[n]), using
sum_e alpha*(fs+fe) = sum_e alpha*T - feat_dst (alpha sums to 1 per dst).
Softmax runs without max-subtraction (scores are O(+-10) here; exp is safe
in fp32).
"""
import numpy as np

import concourse.bacc as bacc
import concourse.bass as bass
import concourse.tile as tile
import concourse.mybir as mybir
from concourse.bass import IndirectOffsetOnAxis
from concourse.bass_utils import run_bass_kernel_spmd

N_NODES = 50000
N_CORES = 8
N_LOCAL = N_NODES // N_CORES          # 6250
IN_FEAT = 128
HEADS = 8
HEAD_DIM = 16
NEG_SLOPE = 0.2
TILE_E = 128                          # edges per tile
TILE_W = 32                           # dst-node window per tile
ST_TILES = 16                         # tiles per super-tile
EDGE_BLK = ST_TILES * TILE_E          # 2048 edge slots per super-tile
EPS_Z = 1e-12                         # DVE reciprocal valid to ~2^-42
P = 128
FP = mybir.dt.float32
BF = mybir.dt.float16
I32 = mybir.dt.int32
BF_NP = np.float16


# ---------------------------------------------------------------- host prep

def _pack_core(dst_local, deg, n_local):
    order = np.argsort(dst_local, kind="stable")
    tile_base, tile_cnt, tile_w = [], [], []
    node_slot = np.full(n_local, -1, np.int64)
    cur_base = cur_cnt = cur_w = 0
    t = 0
    started = False
    empties = []

    def close():
        nonlocal t
        tile_base.append(cur_base)
        tile_cnt.append(cur_cnt)
        tile_w.append(cur_w)
        t += 1

    for n in range(n_local):
        d = int(deg[n])
        if d == 0:
            empties.append(n)
            continue
        assert d <= TILE_E, f"node degree {d} exceeds tile capacity {TILE_E}"
        if not started:
            cur_base, cur_cnt, cur_w = n, 0, 0
            started = True
        if cur_cnt + d > TILE_E or (n - cur_base) >= TILE_W:
            close()
            cur_base, cur_cnt, cur_w = n, 0, 0
        node_slot[n] = t * TILE_W + (n - cur_base)
        cur_w = n - cur_base + 1
        cur_cnt += d
    if started:
        close()

    free_slots = []
    for ti in range(t):
        for s in range(tile_w[ti], TILE_W):
            free_slots.append(ti * TILE_W + s)
    fi = 0
    for n in empties:
        if fi >= len(free_slots):
            tile_base.append(0)
            tile_cnt.append(0)
            tile_w.append(0)
            for s in range(TILE_W):
                free_slots.append(t * TILE_W + s)
            t += 1
        node_slot[n] = free_slots[fi]
        fi += 1
    assert (node_slot >= 0).all()
    return order, np.asarray(tile_base), np.asarray(tile_cnt), node_slot


def _prep_cores(x, efeat, src, dst, W_src, b_src, W_dst, b_dst, W_edge, attn):
    x = np.ascontiguousarray(np.asarray(x, np.float32))
    efeat = np.asarray(efeat, np.float32)
    src = np.asarray(src).astype(np.int64)
    dst = np.asarray(dst).astype(np.int64)
    W_src = np.asarray(W_src, np.float32)
    W_dst = np.asarray(W_dst, np.float32)
    W_edge = np.asarray(W_edge, np.float32)
    b_src = np.asarray(b_src, np.float32)
    b_dst = np.asarray(b_dst, np.float32)
    attn = np.asarray(attn, np.float32)
    has_bias = bool(max(np.abs(b_src).max(), np.abs(b_dst).max()) > 0)

    per_core = []
    core_T = []
    for k in range(N_CORES):
        lo = k * N_LOCAL
        eidx = np.nonzero((dst >= lo) & (dst < lo + N_LOCAL))[0]
        dl = dst[eidx] - lo
        deg = np.bincount(dl, minlength=N_LOCAL)
        order, tb, tcnt, node_slot = _pack_core(dl, deg, N_LOCAL)
        per_core.append((eidx[order], dl[order], tb, tcnt, node_slot))
        core_T.append(len(tb))

    T_tiles = max(core_T)
    T_tiles = ((T_tiles + ST_TILES - 1) // ST_TILES) * ST_TILES
    E_pad = T_tiles * TILE_E

    x16 = x.astype(BF_NP)
    ef16 = efeat.astype(BF_NP)
    attn_rep = np.ascontiguousarray(
        np.broadcast_to(attn.reshape(1, HEADS * HEAD_DIM), (P, HEADS * HEAD_DIM)))
    bias_sum = np.ascontiguousarray(
        np.broadcast_to((b_src + b_dst)[None, :], (P, IN_FEAT)))
    bdst_rep = np.ascontiguousarray(np.broadcast_to(b_dst[None, :], (P, IN_FEAT)))
    n_fin = (N_LOCAL + P - 1) // P * P

    in_maps = []
    for k in range(N_CORES):
        eidx, dl, tb, tcnt, node_slot = per_core[k]
        lo = k * N_LOCAL

        src_ids = np.zeros((TILE_E, T_tiles), np.int64)
        fd_ids = np.zeros((TILE_E, T_tiles), np.int64)
        slot_reb = np.full((TILE_E, T_tiles), -1, np.int64)
        ef_ids = np.full((TILE_E, T_tiles), -1, np.int64)
        pos = 0
        for t in range(len(tb)):
            c = int(tcnt[t])
            if c == 0:
                continue
            e_ids = eidx[pos:pos + c]
            d_loc = dl[pos:pos + c]
            pos += c
            src_ids[:c, t] = src[e_ids]
            fd_ids[:c, t] = d_loc + lo
            slot_reb[:c, t] = d_loc - tb[t]
            ef_ids[:c, t] = e_ids
        assert pos == len(eidx)

        # staged per-edge features, feature-major [128, E_pad], col = t*128+p
        xsrcT = np.ascontiguousarray(x16[src_ids.T.ravel()].T)
        xdstT = np.ascontiguousarray(x16[fd_ids.T.ravel()].T)
        ce = ef_ids.T.ravel()
        efTv = ef16[np.maximum(ce, 0)]
        efTv[ce < 0] = 0
        efTv = np.ascontiguousarray(efTv.T)

        S_all = np.zeros((TILE_E, T_tiles * TILE_W), BF_NP)
        pp, tt_ = np.nonzero(slot_reb >= 0)
        S_all[pp, tt_ * TILE_W + slot_reb[pp, tt_]] = 1.0

        sm = np.zeros(n_fin, np.int32)
        sm[:N_LOCAL] = node_slot.astype(np.int32)
        slot_map = np.ascontiguousarray(sm.reshape(n_fin // P, P).T)

        in_maps.append(dict(
            xT_local=np.ascontiguousarray(x[k * N_LOCAL:(k + 1) * N_LOCAL].T),
            W_dstT32=np.ascontiguousarray(W_dst.T),
            Ws16=np.ascontiguousarray(W_src.T.astype(BF_NP)),
            Wd16=np.ascontiguousarray(W_dst.T.astype(BF_NP)),
            We16=np.ascontiguousarray(W_edge.T.astype(BF_NP)),
            bias_sum=bias_sum,
            bdst_rep=bdst_rep,
            attn_rep=attn_rep,
            xsrcT=xsrcT,
            xdstT=xdstT,
            efT=efTv,
            S_all=S_all,
            slot_map=slot_map,
        ))
    return in_maps, T_tiles, has_bias


# ------------------------------------------------------------- bass program

def _chunks(total, step):
    out = []
    c0 = 0
    while c0 < total:
        out.append((c0, min(step, total - c0)))
        c0 += step
    return out


def build_program(T_tiles, has_bias=False, dbg=False):
    nc = bacc.Bacc("TRN2", target_bir_lowering=False, debug=False,
                   num_devices=N_CORES)
    ikind = "ExternalOutput" if dbg else "Internal"
    n_st = T_tiles // ST_TILES
    n_fin = (N_LOCAL + P - 1) // P * P

    xTl_d = nc.dram_tensor("xT_local", [IN_FEAT, N_LOCAL], FP, kind="ExternalInput")
    WdT32_d = nc.dram_tensor("W_dstT32", [IN_FEAT, IN_FEAT], FP, kind="ExternalInput")
    Ws16_d = nc.dram_tensor("Ws16", [IN_FEAT, IN_FEAT], BF, kind="ExternalInput")
    Wd16_d = nc.dram_tensor("Wd16", [IN_FEAT, IN_FEAT], BF, kind="ExternalInput")
    We16_d = nc.dram_tensor("We16", [IN_FEAT, IN_FEAT], BF, kind="ExternalInput")
    bsum_d = nc.dram_tensor("bias_sum", [P, IN_FEAT], FP, kind="ExternalInput")
    bdst_d = nc.dram_tensor("bdst_rep", [P, IN_FEAT], FP, kind="ExternalInput")
    attn_d = nc.dram_tensor("attn_rep", [P, IN_FEAT], FP, kind="ExternalInput")
    xsT_d = nc.dram_tensor("xsrcT", [IN_FEAT, T_tiles * TILE_E], BF,
                           kind="ExternalInput")
    xdT_d = nc.dram_tensor("xdstT", [IN_FEAT, T_tiles * TILE_E], BF,
                           kind="ExternalInput")
    efT_d = nc.dram_tensor("efT", [IN_FEAT, T_tiles * TILE_E], BF,
                           kind="ExternalInput")
    S_d = nc.dram_tensor("S_all", [TILE_E, T_tiles * TILE_W], BF,
                         kind="ExternalInput")
    smap_d = nc.dram_tensor("slot_map", [P, n_fin // P], I32, kind="ExternalInput")

    fdst_d = nc.dram_tensor("feat_dst_i", [N_LOCAL, IN_FEAT], FP, kind=ikind)
    U_d = nc.dram_tensor("U_i", [T_tiles * TILE_W, IN_FEAT], FP, kind=ikind)
    z_d = nc.dram_tensor("z_i", [T_tiles * TILE_W, HEADS], FP, kind=ikind)
    out_d = nc.dram_tensor("out", [N_LOCAL, IN_FEAT], FP, kind="ExternalOutput")

    HB = EDGE_BLK // 2   # 1024, psum half block

    with tile.TileContext(nc) as tc:
        with tc.tile_pool(name="const", bufs=1) as cb:
            WdT32 = cb.tile([P, IN_FEAT], FP)
            nc.sync.dma_start(out=WdT32[:], in_=WdT32_d[:])
            Ws16 = cb.tile([P, IN_FEAT], BF)
            nc.sync.dma_start(out=Ws16[:], in_=Ws16_d[:])
            Wd16 = cb.tile([P, IN_FEAT], BF)
            nc.sync.dma_start(out=Wd16[:], in_=Wd16_d[:])
            We16 = cb.tile([P, IN_FEAT], BF)
            nc.sync.dma_start(out=We16[:], in_=We16_d[:])
            bsum = cb.tile([P, IN_FEAT], FP)
            nc.sync.dma_start(out=bsum[:], in_=bsum_d[:])
            bdst = cb.tile([P, IN_FEAT], FP)
            nc.sync.dma_start(out=bdst[:], in_=bdst_d[:])
            attn_sb = cb.tile([P, IN_FEAT], FP)
            nc.sync.dma_start(out=attn_sb[:], in_=attn_d[:])
            smap_sb = cb.tile([P, n_fin // P], I32)
            nc.sync.dma_start(out=smap_sb[:], in_=smap_d[:])

            # ---------------- phase A: feat_dst projection (+ b_dst)
            with (
                tc.tile_pool(name="pa_sb", bufs=3) as pas,
                tc.tile_pool(name="pa_ps", bufs=4, space="PSUM") as pap,
                tc.tile_pool(name="pa_tr", bufs=1, space="PSUM") as patr,
            ):
                for c0, cw in _chunks(N_LOCAL, 2048):
                    xc = pas.tile([P, 2048], FP, tag="xc")
                    nc.sync.dma_start(out=xc[:, :cw], in_=xTl_d[:, c0:c0 + cw])
                    tr = patr.tile([TILE_W, 1], FP, tag="trash")
                    nc.tensor.matmul(out=tr[:], lhsT=xc[:, :TILE_W],
                                     rhs=xc[:, :1], start=True, stop=True)
                    for j0, jw in _chunks(cw, P):
                        ps = pap.tile([P, IN_FEAT], FP, tag="pa")
                        nc.tensor.matmul(out=ps[:jw, :], lhsT=xc[:, j0:j0 + jw],
                                         rhs=WdT32[:], start=True, stop=True)
                        ev = pas.tile([P, IN_FEAT], FP, tag="ev")
                        nc.vector.tensor_tensor(out=ev[:jw, :], in0=ps[:jw, :],
                                                in1=bdst[:jw, :],
                                                op=mybir.AluOpType.add)
                        nc.sync.dma_start(out=fdst_d[c0 + j0:c0 + j0 + jw, :],
                                          in_=ev[:jw, :])

            with tc.tile_critical():
                nc.all_engine_barrier()

            # ---------------- phase B: edge super-tiles
            with (
                tc.tile_pool(name="eb_sb", bufs=3) as eb,
                tc.tile_pool(name="eb_ps", bufs=2, space="PSUM") as ep,
                tc.tile_pool(name="eb_ps1", bufs=1, space="PSUM") as ep1,
            ):
                for st in range(n_st):
                    e0 = st * EDGE_BLK

                    xsc = eb.tile([P, EDGE_BLK], BF, tag="xsc")
                    nc.sync.dma_start(out=xsc[:], in_=xsT_d[:, e0:e0 + EDGE_BLK])
                    xdc = eb.tile([P, EDGE_BLK], BF, tag="xdc")
                    nc.sync.dma_start(out=xdc[:], in_=xdT_d[:, e0:e0 + EDGE_BLK])
                    efc = eb.tile([P, EDGE_BLK], BF, tag="efc")
                    nc.sync.dma_start(out=efc[:], in_=efT_d[:, e0:e0 + EDGE_BLK])
                    S_sb = eb.tile([TILE_E, ST_TILES * TILE_W], BF, tag="S")
                    nc.sync.dma_start(
                        out=S_sb[:],
                        in_=S_d[:, st * ST_TILES * TILE_W:(st + 1) * ST_TILES * TILE_W])

                    # wait-absorbing dummies (PE matmuls allow only one wait)
                    tre = ep1.tile([TILE_W, 1], FP, tag="trash")
                    nc.tensor.matmul(out=tre[:], lhsT=xsc[:, :TILE_W],
                                     rhs=xsc[:, :1], start=True, stop=True)
                    nc.tensor.matmul(out=tre[:], lhsT=xdc[:, :TILE_W],
                                     rhs=xdc[:, :1], start=True, stop=True)
                    nc.tensor.matmul(out=tre[:], lhsT=efc[:, :TILE_W],
                                     rhs=efc[:, :1], start=True, stop=True)
                    nc.tensor.matmul(out=tre[:], lhsT=S_sb[:, :TILE_W],
                                     rhs=S_sb[:, :1], start=True, stop=True)

                    # T = fs + fd + fe into PSUM (3 accumulating matmuls/tile)
                    halves = []
                    for h in range(2):
                        psh = ep.tile([P, HB], FP, tag="ps")
                        for tt in range(8):
                            c = (h * 8 + tt) * TILE_E
                            sl = slice(tt * TILE_E, (tt + 1) * TILE_E)
                            nc.tensor.matmul(out=psh[:, sl], lhsT=xsc[:, c:c + TILE_E],
                                             rhs=Ws16[:], start=True, stop=False)
                            nc.tensor.matmul(out=psh[:, sl], lhsT=xdc[:, c:c + TILE_E],
                                             rhs=Wd16[:], start=False, stop=False)
                            nc.tensor.matmul(out=psh[:, sl], lhsT=efc[:, c:c + TILE_E],
                                             rhs=We16[:], start=False, stop=True)
                        halves.append(psh)

                    if has_bias:
                        Tt = eb.tile([P, EDGE_BLK], FP, tag="Tt")
                        for h in range(2):
                            hs = slice(h * HB, (h + 1) * HB)
                            nc.vector.tensor_tensor(
                                out=Tt[:, hs].rearrange("p (t f) -> p t f", t=8),
                                in0=halves[h][:].rearrange("p (t f) -> p t f", t=8),
                                in1=bsum[:].unsqueeze(1).to_broadcast([P, 8, IN_FEAT]),
                                op=mybir.AluOpType.add)
                        tsrc = [Tt[:, :HB], Tt[:, HB:]]
                    else:
                        tsrc = [halves[0][:], halves[1][:]]

                    # leaky-relu then * attn
                    T2 = eb.tile([P, EDGE_BLK], BF, tag="T2")
                    for h in range(2):
                        hs = slice(h * HB, (h + 1) * HB)
                        nc.scalar.activation(out=T2[:, hs], in_=tsrc[h],
                                             func=mybir.ActivationFunctionType.Copy,
                                             scale=NEG_SLOPE)
                        nc.vector.tensor_tensor(out=T2[:, hs], in0=tsrc[h],
                                                in1=T2[:, hs],
                                                op=mybir.AluOpType.max)
                    attn_b = attn_sb[:].unsqueeze(1).to_broadcast(
                        [P, ST_TILES, IN_FEAT])
                    nc.vector.tensor_tensor(
                        out=T2[:].rearrange("p (t f) -> p t f", t=ST_TILES),
                        in0=T2[:].rearrange("p (t f) -> p t f", t=ST_TILES),
                        in1=attn_b, op=mybir.AluOpType.mult)

                    score = eb.tile([P, ST_TILES * HEADS], FP, tag="score")
                    nc.vector.tensor_reduce(
                        out=score[:],
                        in_=T2[:].rearrange("p (t h d) -> p t h d",
                                            h=HEADS, d=HEAD_DIM),
                        axis=mybir.AxisListType.X, op=mybir.AluOpType.add)
                    ex = eb.tile([P, ST_TILES * HEADS], BF, tag="ex")
                    nc.scalar.activation(out=ex[:], in_=score[:],
                                         func=mybir.ActivationFunctionType.Exp)

                    # msg = T * ex  (bf16)
                    msg = eb.tile([P, EDGE_BLK], BF, tag="msg")
                    for h in range(2):
                        hs = slice(h * HB, (h + 1) * HB)
                        ex_b = ex[:, h * 64:(h + 1) * 64] \
                            .rearrange("p (t hh) -> p t hh", hh=HEADS) \
                            .unsqueeze(3).to_broadcast([P, 8, HEADS, HEAD_DIM])
                        nc.vector.tensor_tensor(
                            out=msg[:, hs].rearrange("p (t hh d) -> p t hh d",
                                                     hh=HEADS, d=HEAD_DIM),
                            in0=tsrc[h].rearrange("p (t hh d) -> p t hh d",
                                                  hh=HEADS, d=HEAD_DIM),
                            in1=ex_b, op=mybir.AluOpType.mult)

                    # absorb DVE wait before scatter matmuls
                    nc.tensor.matmul(out=tre[:], lhsT=msg[:, :TILE_W],
                                     rhs=msg[:, :1], start=True, stop=True)

                    U_sb = eb.tile([TILE_W, ST_TILES * IN_FEAT], FP, tag="Usb")
                    z_ps = ep1.tile([TILE_W, ST_TILES * HEADS], FP, tag="zps")
                    for q in range(4):
                        U_ps = ep.tile([TILE_W, 4 * IN_FEAT], FP, tag="Ups")
                        for j in range(4):
                            tt = q * 4 + j
                            nc.tensor.matmul(
                                out=U_ps[:, j * IN_FEAT:(j + 1) * IN_FEAT],
                                lhsT=S_sb[:, tt * TILE_W:(tt + 1) * TILE_W],
                                rhs=msg[:, tt * TILE_E:(tt + 1) * TILE_E],
                                start=True, stop=True)
                            nc.tensor.matmul(
                                out=z_ps[:, tt * HEADS:(tt + 1) * HEADS],
                                lhsT=S_sb[:, tt * TILE_W:(tt + 1) * TILE_W],
                                rhs=ex[:, tt * HEADS:(tt + 1) * HEADS],
                                start=True, stop=True)
                        nc.scalar.activation(
                            out=U_sb[:, q * 512:(q + 1) * 512], in_=U_ps[:],
                            func=mybir.ActivationFunctionType.Copy)
                    z_sb = eb.tile([TILE_W, ST_TILES * HEADS], FP, tag="zsb")
                    nc.scalar.activation(out=z_sb[:], in_=z_ps[:],
                                         func=mybir.ActivationFunctionType.Copy)

                    nc.sync.dma_start(
                        out=U_d[st * ST_TILES * TILE_W:(st + 1) * ST_TILES * TILE_W, :]
                        .rearrange("(t w) f -> w t f", t=ST_TILES),
                        in_=U_sb[:].rearrange("p (t f) -> p t f", t=ST_TILES))
                    nc.sync.dma_start(
                        out=z_d[st * ST_TILES * TILE_W:(st + 1) * ST_TILES * TILE_W, :]
                        .rearrange("(t w) h -> w t h", t=ST_TILES),
                        in_=z_sb[:].rearrange("p (t h) -> p t h", t=ST_TILES))

            with tc.tile_critical():
                nc.all_engine_barrier()

            # ---------------- phase C: normalize, subtract feat_dst, relu
            with tc.tile_pool(name="fin", bufs=3) as fb:
                for i, (c0, w) in enumerate(_chunks(N_LOCAL, P)):
                    Ug = fb.tile([P, IN_FEAT], FP, tag="Ug")
                    nc.gpsimd.indirect_dma_start(
                        out=Ug[:w, :], out_offset=None, in_=U_d[:],
                        in_offset=IndirectOffsetOnAxis(
                            ap=smap_sb[:w, i:i + 1], axis=0))
                    zg = fb.tile([P, HEADS], FP, tag="zg")
                    nc.gpsimd.indirect_dma_start(
                        out=zg[:w, :], out_offset=None, in_=z_d[:],
                        in_offset=IndirectOffsetOnAxis(
                            ap=smap_sb[:w, i:i + 1], axis=0))
                    fdr = fb.tile([P, IN_FEAT], FP, tag="fdr")
                    nc.sync.dma_start(out=fdr[:w, :], in_=fdst_d[c0:c0 + w, :])

                    zs = fb.tile([P, HEADS], FP, tag="zs")
                    nc.vector.tensor_scalar_max(out=zs[:w, :], in0=zg[:w, :],
                                                scalar1=EPS_Z)
                    zr = fb.tile([P, HEADS], FP, tag="zr")
                    nc.vector.reciprocal(out=zr[:w, :], in_=zs[:w, :])
                    m = fb.tile([P, HEADS], FP, tag="m")
                    nc.vector.tensor_scalar(out=m[:w, :], in0=zg[:w, :],
                                            scalar1=0.0, scalar2=None,
                                            op0=mybir.AluOpType.is_gt)
                    mz = fb.tile([P, HEADS], FP, tag="mz")
                    nc.vector.tensor_tensor(out=mz[:w, :], in0=zr[:w, :],
                                            in1=m[:w, :], op=mybir.AluOpType.mult)
                    hp = fb.tile([P, IN_FEAT], FP, tag="hp")
                    mz_b = mz[:w, :].unsqueeze(2).to_broadcast([w, HEADS, HEAD_DIM])
                    nc.vector.tensor_tensor(
                        out=hp[:w, :].rearrange("p (h d) -> p h d", d=HEAD_DIM),
                        in0=Ug[:w, :].rearrange("p (h d) -> p h d", d=HEAD_DIM),
                        in1=mz_b, op=mybir.AluOpType.mult)
                    fdm = fb.tile([P, IN_FEAT], FP, tag="fdm")
                    m_b = m[:w, :].unsqueeze(2).to_broadcast([w, HEADS, HEAD_DIM])
                    nc.vector.tensor_tensor(
                        out=fdm[:w, :].rearrange("p (h d) -> p h d", d=HEAD_DIM),
                        in0=fdr[:w, :].rearrange("p (h d) -> p h d", d=HEAD_DIM),
                        in1=m_b, op=mybir.AluOpType.mult)
                    h2 = fb.tile([P, IN_FEAT], FP, tag="h2")
                    nc.vector.tensor_tensor(out=h2[:w, :], in0=hp[:w, :],
                                            in1=fdm[:w, :],
                                            op=mybir.AluOpType.subtract)
                    ob = fb.tile([P, IN_FEAT], FP, tag="ob")
                    nc.scalar.activation(out=ob[:w, :], in_=h2[:w, :],
                                         func=mybir.ActivationFunctionType.Relu)
                    nc.sync.dma_start(out=out_d[c0:c0 + w, :], in_=ob[:w, :])
    nc.compile()
    return nc


_PROGRAM_CACHE = {}


def kernel(**inputs) -> np.ndarray:
    in_maps, T_tiles, has_bias = _prep_cores(**inputs)
    key = (T_tiles, has_bias)
    if key not in _PROGRAM_CACHE:
        _PROGRAM_CACHE[key] = build_program(T_tiles, has_bias=has_bias)
    nc = _PROGRAM_CACHE[key]
    res = run_bass_kernel_spmd(nc, in_maps, list(range(N_CORES)))
    out = np.concatenate([np.asarray(res.results[k]["out"])
                          for k in range(N_CORES)], axis=0)
    return out.astype(np.float32)


if __name__ == "__main__":
    from prep import load_inputs_npz, reference_np
    inputs = load_inputs_npz()
    actual = kernel(**inputs)
    ref_in = {k: (v.astype(np.int64) if k in ("src", "dst")
                  else np.asarray(v, np.float32)) for k, v in inputs.items()}
    expected = reference_np(**ref_in)
    rel = np.linalg.norm(actual - expected) / np.linalg.norm(expected)
    print(f"rel l2 err: {rel:.3e}  max abs: {np.abs(actual - expected).max():.3e}")
